# revision 1
# baseline (speedup 1.0000x reference)
"""Trainium2 Bass kernel for brute-force kNN (nn_BruteForce_72541997629642).

Problem: queries [256, 64] f32, candidates [1e6, 64] f32, ids [1e6] i32,
k=10.  reference: scores = queries @ candidates.T; top_k(scores, k).

Device (per core, 125k candidates padded to 2*W, bf16 matmul):
  - host pre-transposes + bf16-casts the candidate shard into cT [128, W]:
    rows 0:64 = dims of candidates [0, W) (chunk A), rows 64:128 = dims of
    candidates [W, 2W) (chunk B); queries stationary [64, 128] per group.
  - TensorE: bf16 matmuls, 512 moving cols each; two fill one [128, 1024]
    PSUM tile (2 banks).  4 PSUM buffers (8 banks) so the PE can fill two
    tiles ahead while BOTH drain engines work concurrently - with one
    2-buffer tag the DVE and ACT drains serialize (measured 276 us vs
    161 us for this build).
  - Drain: alternating tiles on two independent lanes:
      D tiles -> DVE reduce_max of 32-blocks from PSUM -> fp32 block-max
                 table bm [128, 3968],
      A tiles -> ACT relu(s - t_q) (per-query threshold via the
                 per-partition bias operand) with accum_out = row sum =
                 "any candidate above t_q" flag -> fl [128, 124].
    Lane rates ~1.19/1.18 us per 1024-col tile -> drain-bound ~148 us;
    measured 161 us/iter (vs 210-265 us for the previous cast+fold kernel).

Host:
  - t_host[q] = z * ||q|| with z s.t. E[#candidates above] ~= 50 >> k;
    device threshold t_dev = t_host - 0.3 (bf16 score noise is ~0.035).
  - gather D blocks with bm > t_dev (32 cands) + A tiles with flag > 0
    (1024 cands); exact fp32 rescore of the union; exact top-k with
    jax.lax.top_k tie order (lower index first).
  - per-query safety: if fewer than k exact scores above t_host, fall back
    to a full exact rescore of that query (never triggers on the reference
    seed; guarantees correctness regardless of device-side noise).
"""

from contextlib import ExitStack, nullcontext

import ml_dtypes
import numpy as np

import concourse.bass as bass
import concourse.mybir as mybir
import concourse.tile as tile
from concourse.bass_utils import run_bass_kernel_spmd

f32 = mybir.dt.float32
bf16 = mybir.dt.bfloat16
f8 = mybir.dt.float8e4
FP8 = ml_dtypes.float8_e4m3

# ---------------- problem constants (hardcoded per spec) ----------------
B = 256          # queries
D = 64           # dims
N = 1_000_000    # candidates
NCORES = 8
NSHARD = N // NCORES          # 125000 candidates per core
W = 63488                     # chunk width (= 31 * 2048)
NPAD = 2 * W                  # padded per-core candidates (126976)
BS = 32                       # block size for block-max
F_DMA = 2048                  # candidate columns per DMA tile
F_MM = 512                    # moving free dim per matmul (1 PSUM bank)
F_PS = 1024                   # PSUM tile columns (2 banks; 4 bufs = 8 banks)
N_DMA_TILES = W // F_DMA      # 31
N_TILES = N_DMA_TILES * 2 * 2 * (F_DMA // F_PS)  # 248 psum tiles/core

Z_COUNT = 50.0                # target E[#cands above t_host] (>= 3k+20)
DEV_MARGIN = 0.3              # t_dev = t_host - margin (bf16 noise ~8 sigma)

_MAX_WAITS = 1


def _z_from_count(count: float) -> float:
    from math import erf, sqrt

    lo, hi = 0.0, 8.0
    target = count / 1e6
    for _ in range(80):
        mid = (lo + hi) / 2
        p = 0.5 * (1 - erf(mid / sqrt(2)))
        if p > target:
            lo = mid
        else:
            hi = mid
    return (lo + hi) / 2


def schedule():
    """Per-PSUM-tile drain plan shared by device builder and host decode.

    Tile order: ti (31) x g (2) x h (2) x half (2 x 1024 cols).
    Returns (sched, n_bm_slots, n_flags); sched entries:
      kind 'D'/'A', g, h, base (chunk-local candidate base), off.
    """
    sched = []
    d_off = 0
    a_idx = 0
    idx = 0
    for ti in range(N_DMA_TILES):
        for g in range(2):
            for h in range(2):
                for sub in range(F_DMA // F_PS):
                    base = ti * F_DMA + sub * F_PS
                    if idx % 2 == 0:
                        sched.append(
                            dict(kind="D", g=g, h=h, base=base, off=d_off)
                        )
                        d_off += F_PS // BS
                    else:
                        sched.append(
                            dict(kind="A", g=g, h=h, base=base, off=a_idx)
                        )
                        a_idx += 1
                    idx += 1
    return sched, d_off, a_idx


SCHED, N_BM_SLOTS, N_FLAGS = schedule()


def _split_excess_waits(nc):
    n_nops = 0
    for f in nc.m.functions:
        for bb in f.blocks:
            new_insts = []
            dirty = False
            for ins in bb.instructions:
                si = ins.sync_info
                if (
                    si is not None
                    and si.on_wait is not None
                    and len(si.on_wait) > _MAX_WAITS
                ):
                    waits = list(si.on_wait)
                    keep = waits[: _MAX_WAITS]
                    rest = waits[_MAX_WAITS:]
                    for j in range(0, len(rest), _MAX_WAITS):
                        nop = mybir.InstNoOp(name=f"I-waitsplit-{n_nops}")
                        n_nops += 1
                        nop.engine = ins.engine
                        nop.sync_info = mybir.SyncInfo(
                            on_wait=rest[j : j + _MAX_WAITS], on_update=[]
                        )
                        new_insts.append(nop)
                    ins.sync_info = mybir.SyncInfo(
                        on_wait=keep, on_update=list(si.on_update or [])
                    )
                    dirty = True
                new_insts.append(ins)
            if dirty:
                bb.instructions = new_insts
    return n_nops


def _build_nc(repeat: int = 1, loop_repeat: int = 0, mode: str = "full",
              cpool_bufs: int = 3, psum_bufs: int = 4):
    """mode: 'full' | 'mm' (matmul only) | 'dve' | 'act' (single-lane)."""
    nc = bass.Bass()
    qT = nc.dram_tensor("qT", [128, B], bf16, kind="ExternalInput")
    cT = nc.dram_tensor("cT", [128, W], bf16, kind="ExternalInput")
    negt = nc.dram_tensor("negt", [128, 2], f32, kind="ExternalInput")
    n_bm = N_BM_SLOTS if mode != "dve" else N_TILES * (F_PS // BS)
    n_fl = N_FLAGS if mode != "act" else N_TILES
    bm = nc.dram_tensor("bm", [128, n_bm], f32, kind="ExternalOutput")
    fl = nc.dram_tensor("fl", [128, n_fl], f32, kind="ExternalOutput")

    with tile.TileContext(nc) as tc, ExitStack() as ctx:
        qpool = ctx.enter_context(tc.tile_pool(name="qpool", bufs=1))
        cpool = ctx.enter_context(tc.tile_pool(name="cpool", bufs=cpool_bufs))
        pp = ctx.enter_context(tc.tile_pool(name="pp", bufs=psum_bufs, space="PSUM"))
        bmp = ctx.enter_context(tc.tile_pool(name="bmp", bufs=1))
        scp = ctx.enter_context(tc.tile_pool(name="scp", bufs=2))

        qt = qpool.tile([128, B], bf16)
        nc.sync.dma_start(out=qt[:], in_=qT[:])
        ngt = qpool.tile([128, 2], f32)
        nc.sync.dma_start(out=ngt[:], in_=negt[:])
        bm_sb = bmp.tile([128, n_bm], f32, name="bmsb", tag="bmsb")
        fl_sb = bmp.tile([128, n_fl], f32, name="flsb", tag="flsb")
        if mode in ("mm", "dve", "act"):
            nc.gpsimd.memset(bm_sb[:], -3.0e38)
            nc.gpsimd.memset(fl_sb[:], 0.0)

        def body_ctx():
            if loop_repeat > 0:
                return tc.For_i(
                    0,
                    loop_repeat,
                    1,
                    hint_engines=(
                        mybir.EngineType.PE,
                        mybir.EngineType.DVE,
                        mybir.EngineType.SP,
                        mybir.EngineType.Activation,
                    ),
                )
            return nullcontext()

        with body_ctx():
            for _rep in range(repeat):
                d_off = 0
                a_idx = 0
                idx = 0
                for ti in range(N_DMA_TILES):
                    ct = cpool.tile([128, F_DMA], bf16)
                    nc.sync.dma_start(
                        out=ct[:], in_=cT[:, ti * F_DMA : (ti + 1) * F_DMA]
                    )
                    for g in range(2):
                        for h in range(2):
                            for half in range(F_DMA // F_PS):
                                ps = pp.tile([128, F_PS], f32, name="ps", tag="ps")
                                for sub in range(F_PS // F_MM):
                                    j0 = half * F_PS + sub * F_MM
                                    nc.tensor.matmul(
                                        out=ps[:, F_MM * sub : F_MM * (sub + 1)],
                                        lhsT=qt[
                                            64 * h : 64 * (h + 1),
                                            128 * g : 128 * (g + 1),
                                        ],
                                        rhs=ct[
                                            64 * h : 64 * (h + 1),
                                            j0 : j0 + F_MM,
                                        ],
                                        start=True,
                                        stop=True,
                                    )
                                if mode == "mm":
                                    idx += 1
                                    continue
                                use_dve = (
                                    (idx % 2 == 0) if mode == "full"
                                    else (mode == "dve")
                                )
                                if use_dve:
                                    nc.vector.reduce_max(
                                        out=bm_sb[:, d_off : d_off + F_PS // BS],
                                        in_=ps[:].rearrange(
                                            "p (nb bs) -> p nb bs", bs=BS
                                        ),
                                        axis=mybir.AxisListType.X,
                                    )
                                    d_off += F_PS // BS
                                else:
                                    scr = scp.tile(
                                        [128, F_PS], bf16, name="scr", tag="scr"
                                    )
                                    nc.scalar.activation(
                                        out=scr[:],
                                        in_=ps[:],
                                        func=mybir.ActivationFunctionType.Relu,
                                        bias=ngt[:, g : g + 1],
                                        scale=1.0,
                                        accum_out=fl_sb[:, a_idx : a_idx + 1],
                                    )
                                    a_idx += 1
                                idx += 1
        nc.sync.dma_start(out=bm[:], in_=bm_sb[:])
        nc.sync.dma_start(out=fl[:], in_=fl_sb[:])
    _split_excess_waits(nc)
    nc.finalize()
    return nc


_NC_CACHE: dict[tuple, object] = {}


def get_nc(loop_repeat: int = 1, mode: str = "full"):
    key = (loop_repeat, mode)
    if key not in _NC_CACHE:
        _NC_CACHE[key] = _build_nc(repeat=1, loop_repeat=loop_repeat, mode=mode)
    return _NC_CACHE[key]


def _thresholds(queries: np.ndarray, k: int):
    qn = np.linalg.norm(np.asarray(queries, np.float32), axis=1)
    z_h = _z_from_count(max(3 * k + 20, Z_COUNT))
    t_host = z_h * qn
    t_dev = t_host - DEV_MARGIN
    return t_host, t_dev


def _prep_inputs(queries: np.ndarray, candidates: np.ndarray, k: int = 10):
    q = np.asarray(queries, dtype=np.float32)
    c = np.asarray(candidates, dtype=np.float32)
    qT = np.ascontiguousarray(q.T)  # [64, 256]
    qT2 = np.concatenate([qT, qT], axis=0).astype(ml_dtypes.bfloat16)
    _, t_dev = _thresholds(q, k)
    negt = np.zeros((128, 2), dtype=np.float32)
    for g in range(2):
        negt[:, g] = -t_dev[g * 128 : (g + 1) * 128]
    in_maps = []
    for core in range(NCORES):
        shard = c[core * NSHARD : (core + 1) * NSHARD]
        half_a = shard[:W]
        half_b = shard[W:]
        cT2 = np.zeros((128, W), dtype=ml_dtypes.bfloat16)
        cT2[:D, :] = half_a.T.astype(ml_dtypes.bfloat16)
        cT2[D:, : half_b.shape[0]] = half_b.T.astype(ml_dtypes.bfloat16)
        in_maps.append({"qT": qT2, "cT": cT2, "negt": negt})
    return in_maps


def _host_finish(bm_all, fl_all, queries, candidates, ids, k):
    q = np.asarray(queries, dtype=np.float32)
    c = np.asarray(candidates, dtype=np.float32)
    ids = np.asarray(ids)
    k = int(k)
    t_host, t_dev = _thresholds(q, k)

    slot_g = np.empty(N_BM_SLOTS, dtype=np.int64)
    slot_base = np.empty(N_BM_SLOTS, dtype=np.int64)
    flag_g = np.empty(N_FLAGS, dtype=np.int64)
    flag_base = np.empty(N_FLAGS, dtype=np.int64)
    for s in SCHED:
        base = s["h"] * W + s["base"]
        if s["kind"] == "D":
            nsl = F_PS // BS
            slot_g[s["off"] : s["off"] + nsl] = s["g"]
            slot_base[s["off"] : s["off"] + nsl] = base + BS * np.arange(nsl)
        else:
            flag_g[s["off"]] = s["g"]
            flag_base[s["off"]] = base

    cand_lists: list[list[np.ndarray]] = [[] for _ in range(B)]
    for g in range(2):
        tdv = t_dev[g * 128 : (g + 1) * 128]
        sg = np.where(slot_g == g)[0]
        selD = bm_all[:, :, sg] > tdv[None, :, None]
        coreD, pD, siD = np.nonzero(selD)
        baseD = slot_base[sg[siD]] + coreD * NPAD
        fg = np.where(flag_g == g)[0]
        selA = fl_all[:, :, fg] > 0.0
        coreA, pA, fiA = np.nonzero(selA)
        baseA = flag_base[fg[fiA]] + coreA * NPAD
        for p in range(128):
            b = g * 128 + p
            mD = pD == p
            mA = pA == p
            parts = []
            if mD.any():
                bb = baseD[mD]
                parts.append((bb[:, None] + np.arange(BS)[None, :]).ravel())
            if mA.any():
                bb = baseA[mA]
                parts.append((bb[:, None] + np.arange(F_PS)[None, :]).ravel())
            cand_lists[b] = parts

    top_scores = np.empty((B, k), dtype=np.float32)
    top_idx = np.empty((B, k), dtype=np.int32)
    n_fallback = 0
    for b in range(B):
        parts = cand_lists[b]
        if not parts:
            sc_all = c @ q[b]
            n_fallback += 1
            ss, si = _topk_exact(sc_all, np.arange(N), k)
            top_scores[b] = ss
            top_idx[b] = ids[si]
            continue
        lidx = np.concatenate(parts)
        core = lidx // NPAD
        local = lidx - core * NPAD
        valid = local < NSHARD
        gi = np.unique(core[valid] * NSHARD + local[valid])
        sc = c[gi] @ q[b]
        if (sc > t_host[b]).sum() < k:
            sc_all = c @ q[b]
            n_fallback += 1
            ss, si = _topk_exact(sc_all, np.arange(N), k)
        else:
            ss, si = _topk_exact(sc, gi, k)
        top_scores[b] = ss
        top_idx[b] = ids[si]
    if n_fallback:
        print(f"[kernel] host fallback full-rescore for {n_fallback} queries")
    return top_scores, top_idx


def _topk_exact(scores: np.ndarray, idxs: np.ndarray, k: int):
    m = min(4 * k, len(scores) - 1)
    part = np.argpartition(-scores, m)[: m + 1]
    order = np.lexsort((idxs[part], -scores[part]))
    sel = part[order[:k]]
    return scores[sel], idxs[sel]


def kernel(queries, candidates, ids, k):
    k = int(k)
    in_maps = _prep_inputs(queries, candidates, k)
    nc = get_nc(loop_repeat=1, mode="full")
    res = run_bass_kernel_spmd(nc, in_maps, core_ids=list(range(NCORES)))
    bm_all = np.stack([res.results[c]["bm"] for c in range(NCORES)])
    fl_all = np.stack([res.results[c]["fl"] for c in range(NCORES)])
    return _host_finish(
        bm_all,
        fl_all,
        np.asarray(queries, np.float32),
        np.asarray(candidates, np.float32),
        np.asarray(ids),
        k,
    )

